# revision 7
# baseline (speedup 1.0000x reference)
"""Contrastive-loss kernel for trn2 (8 NeuronCores, SPMD).

The reference loss reduces to a Gram matrix G = F.T @ F over the
flattened input F [N=524288, T=64] (128 MiB fp32), followed by a tiny
[64,64] masked margin reduction.  Each core streams a contiguous
row-shard of F (16 MiB) through SBUF as 8 tiles of 8192 rows (16 KiB
contiguous read / 8 KiB bf16 write per partition per tile -- large
descriptors amortize the per-packet cost of the bank's slow edge DMA
engine), casting fp32->bf16 inline in the SWDGE DMA, and accumulates
chunk.T @ chunk matmuls (K=128, M=N=64) into one PSUM accumulator
(fp32).  The 8 partial [64,64] Grams are summed on the host, where
the masked margin reduction (negligible work) also runs.

Raw bacc (no TileContext): the kernel is a simple 3-stage pipeline
(DMA -> PE -> copy/out), and Tile's fixed preamble + end-of-kernel
drain/barrier/sem-clear machinery costs ~19us on a ~50us kernel.
Semaphore protocol:
  - dma_sem[k] (k = slot index): SWDGE incs by 16 per completed input
    DMA into slot k; PE waits 16*(round+1) before consuming.  Per-slot
    sems make the wait robust to cross-DMA completion interleaving
    (sem counts are cumulative across DMAs on one queue).
  - pe_sem: PE incs 1 on the last matmul of each tile; the DMA engine
    waits pe_sem >= i-NBUF+1 before overwriting slot i%NBUF.
  - out_sem: PE-done -> DVE copies PSUM->SBUF -> incs; sync engine
    waits, stores the [64,64] result, incs fin_sem by 16.
  - gpsimd waits fin_sem, then resets DMA state and clears all sems so
    the NEFF can be re-executed (sems must be 0 at kernel entry).
"""

import numpy as np

import concourse.bacc as bacc
import concourse.mybir as mybir
from concourse.bass_utils import run_bass_kernel_spmd

MARGIN = 60000.0
S = 64                      # time steps (Gram dim)
N_TOTAL = 2 * 8 * 32 * 32 * 32   # 524288 flattened rows
N_CORES = 8
N_SHARD = N_TOTAL // N_CORES     # 65536 rows per core
P = 128                     # SBUF partitions
ROWS_PER_TILE = 8192        # rows per DMA tile: 2 MiB fp32 read, 1 MiB bf16
RPP = ROWS_PER_TILE // P    # rows per partition within a tile (64)
FREE = RPP * S              # free dim of an input tile (4096)
CHUNKS = FREE // S          # matmul chunks per tile (64)
N_TILES = N_SHARD // ROWS_PER_TILE   # 8 DMA tiles per core
NBUF = 8                    # slots == tiles: no reuse, no flow control

_CACHE = {}
LAST_RESULTS = None         # BassKernelResults of the most recent run


def _build_nc():
    nc = bacc.Bacc("TRN2", target_bir_lowering=False, debug=False,
                   num_devices=N_CORES)
    x = nc.dram_tensor("x", [N_SHARD, S], mybir.dt.float32,
                       kind="ExternalInput")
    g = nc.dram_tensor("g", [S, S], mybir.dt.float32, kind="ExternalOutput")
    xv = x[:].rearrange("(n p r) c -> n p (r c)", p=P, r=RPP)

    with (
        nc.sbuf_tensor("xbuf", [P, NBUF * FREE], mybir.dt.bfloat16) as xbuf,
        nc.psum_tensor("acc", [2 * S, 2 * S], mybir.dt.float32) as acc,
        nc.sbuf_tensor("obuf", [S, S], mybir.dt.float32) as obuf,
        nc.semaphore("pe_sem") as pe_sem,
        nc.semaphore("out_sem") as out_sem,
        nc.semaphore("fin_sem") as fin_sem,
    ):
        dma_sems = []
        import contextlib
        with contextlib.ExitStack() as stack:
            for k in range(NBUF):
                dma_sems.append(stack.enter_context(
                    nc.semaphore(f"dma_sem{k}")))
            all_sems = [pe_sem, out_sem, fin_sem] + dma_sems

            with nc.Block() as block:

                @block.gpsimd
                def _(gp):
                    for i in range(N_TILES):
                        k = i % NBUF
                        if i >= NBUF:
                            gp.wait_ge(pe_sem, i - NBUF + 1)
                        gp.dma_start(
                            xbuf[:, k * FREE:(k + 1) * FREE], xv[i]
                        ).then_inc(dma_sems[k], 16)
                    # Teardown, split to overlap with the output path: once
                    # PE has consumed every tile the input-DMA sems and
                    # pe_sem are quiescent, so clear them while DVE/SP run
                    # the copy+store; only out/fin teardown needs the store
                    # to have landed.
                    gp.wait_ge(pe_sem, N_TILES)
                    lo = min(s.num for s in dma_sems)
                    hi = max(s.num for s in dma_sems)
                    assert hi - lo == NBUF - 1, (lo, hi)
                    gp.sem_clear(range(lo, hi + 1))
                    gp.sem_clear(pe_sem)
                    gp.wait_ge(fin_sem, 16)
                    gp.dma_reset()
                    gp.sem_clear(out_sem)
                    gp.sem_clear(fin_sem)

                @block.tensor
                def _(te):
                    # Pack 2 row-chunks per matmul: lhsT = rhs = [A|B]
                    # ([128, 128] bf16 -> FWL kicks in), accumulating
                    # [[A'A, A'B], [B'A, B'B]] into a [128,128] PSUM tile.
                    # The two diagonal 64x64 blocks sum to the Gram
                    # contribution; off-diagonal blocks are discarded.
                    for i in range(N_TILES):
                        k = i % NBUF
                        te.wait_ge(dma_sems[k], 16 * (i // NBUF + 1))
                        for j in range(CHUNKS // 2):
                            c = xbuf[:, k * FREE + j * 2 * S:
                                     k * FREE + (j + 1) * 2 * S]
                            mm = te.matmul(
                                acc[:], c, c,
                                start=(i == 0 and j == 0),
                                stop=(i == N_TILES - 1 and j == CHUNKS // 2 - 1),
                            )
                            if j == CHUNKS // 2 - 1:
                                mm.then_inc(pe_sem, 1)

                @block.vector
                def _(v):
                    v.wait_ge(pe_sem, N_TILES)
                    v.tensor_copy(obuf[:], acc[:S, :S])
                    v.tensor_add(obuf[:], obuf[:],
                                 acc[S:, S:]).then_inc(out_sem, 1)

                @block.sync
                def _(sy):
                    sy.wait_ge(out_sem, 1)
                    sy.dma_start(g[:], obuf[:]).then_inc(fin_sem, 16)

    nc.compile()
    return nc


def get_nc():
    if "nc" not in _CACHE:
        _CACHE["nc"] = _build_nc()
    return _CACHE["nc"]


def _device_partial_grams(flat: np.ndarray, **run_kwargs) -> np.ndarray:
    """Run the SPMD bass kernel; return the 8 partial Grams [8, 64, 64]."""
    global LAST_RESULTS
    nc = get_nc()
    in_maps = [
        {"x": flat[c * N_SHARD:(c + 1) * N_SHARD]} for c in range(N_CORES)
    ]
    LAST_RESULTS = run_bass_kernel_spmd(
        nc, in_maps, core_ids=list(range(N_CORES)), **run_kwargs
    )
    return np.stack([LAST_RESULTS.results[c]["g"] for c in range(N_CORES)])


def kernel(input: np.ndarray, **run_kwargs) -> np.ndarray:
    flat = np.ascontiguousarray(
        np.asarray(input, dtype=np.float32).reshape(N_TOTAL, S)
    )
    partials = _device_partial_grams(flat, **run_kwargs)

    gram = partials.astype(np.float64).sum(axis=0)
    sq = np.diag(gram)
    dist = sq[:, None] + sq[None, :] - 2.0 * gram
    idx = np.arange(S)
    lower = idx[:, None] > idx[None, :]
    adjacent = (idx[:, None] - idx[None, :]) == 1
    per_pair = np.where(adjacent, np.maximum(0.0, MARGIN - dist), dist)
    loss = np.where(lower, per_pair, 0.0).sum() / (S * (S - 1) * 1000)
    return np.asarray(loss, dtype=np.float32)



# revision 9
# speedup vs baseline: 1.0461x; 1.0461x over previous
"""Contrastive-loss kernel for trn2 (8 NeuronCores, SPMD).

The reference loss reduces to a Gram matrix G = F.T @ F over the
flattened input F [N=524288, T=64] (128 MiB fp32), followed by a tiny
[64,64] masked margin reduction.  Each core streams a contiguous
row-shard of F (16 MiB) through SBUF as 16 tiles of 4096 rows, all
issued up-front into dedicated slots (no reuse, no flow control, so a
lagging DMA engine never throttles the healthy ones through pe_sem),
casting fp32->bf16 inline in the SWDGE DMA, and accumulates
chunk.T @ chunk matmuls (K=128, M=N=64) into one PSUM accumulator
(fp32).  The 8 partial [64,64] Grams are summed on the host, where
the masked margin reduction (negligible work) also runs.

Raw bacc (no TileContext): the kernel is a simple 3-stage pipeline
(DMA -> PE -> copy/out), and Tile's fixed preamble + end-of-kernel
drain/barrier/sem-clear machinery costs ~19us on a ~50us kernel.
Semaphore protocol:
  - dma_sem[k] (k = slot index): SWDGE incs by 16 per completed input
    DMA into slot k; PE waits 16*(round+1) before consuming.  Per-slot
    sems make the wait robust to cross-DMA completion interleaving
    (sem counts are cumulative across DMAs on one queue).
  - pe_sem: PE incs 1 on the last matmul of each tile; the DMA engine
    waits pe_sem >= i-NBUF+1 before overwriting slot i%NBUF.
  - out_sem: PE-done -> DVE copies PSUM->SBUF -> incs; sync engine
    waits, stores the [64,64] result, incs fin_sem by 16.
  - gpsimd waits fin_sem, then resets DMA state and clears all sems so
    the NEFF can be re-executed (sems must be 0 at kernel entry).
"""

import numpy as np

import concourse.bacc as bacc
import concourse.mybir as mybir
from concourse.bass_utils import run_bass_kernel_spmd

MARGIN = 60000.0
S = 64                      # time steps (Gram dim)
N_TOTAL = 2 * 8 * 32 * 32 * 32   # 524288 flattened rows
N_CORES = 8
N_SHARD = N_TOTAL // N_CORES     # 65536 rows per core
P = 128                     # SBUF partitions
ROWS_PER_TILE = 4096        # rows per DMA tile: 1 MiB fp32 read, 512 KiB bf16
RPP = ROWS_PER_TILE // P    # rows per partition within a tile (32)
FREE = RPP * S              # free dim of an input tile (2048)
CHUNKS = FREE // S          # matmul chunks per tile (32)
N_TILES = N_SHARD // ROWS_PER_TILE   # 16 DMA tiles per core
NBUF = 16                   # slots == tiles: no reuse, no flow control

_CACHE = {}
LAST_RESULTS = None         # BassKernelResults of the most recent run


def _build_nc():
    nc = bacc.Bacc("TRN2", target_bir_lowering=False, debug=False,
                   num_devices=N_CORES)
    x = nc.dram_tensor("x", [N_SHARD, S], mybir.dt.float32,
                       kind="ExternalInput")
    g = nc.dram_tensor("g", [S, S], mybir.dt.float32, kind="ExternalOutput")
    xv = x[:].rearrange("(n p r) c -> n p (r c)", p=P, r=RPP)

    with (
        nc.sbuf_tensor("xbuf", [P, NBUF * FREE], mybir.dt.bfloat16) as xbuf,
        nc.psum_tensor("acc", [2 * S, 2 * S], mybir.dt.float32) as acc,
        nc.sbuf_tensor("obuf", [S, S], mybir.dt.float32) as obuf,
        nc.semaphore("pe_sem") as pe_sem,
        nc.semaphore("out_sem") as out_sem,
        nc.semaphore("fin_sem") as fin_sem,
    ):
        dma_sems = []
        import contextlib
        with contextlib.ExitStack() as stack:
            for k in range(NBUF):
                dma_sems.append(stack.enter_context(
                    nc.semaphore(f"dma_sem{k}")))
            all_sems = [pe_sem, out_sem, fin_sem] + dma_sems

            with nc.Block() as block:

                @block.gpsimd
                def _(gp):
                    for i in range(N_TILES):
                        k = i % NBUF
                        if i >= NBUF:
                            gp.wait_ge(pe_sem, i - NBUF + 1)
                        gp.dma_start(
                            xbuf[:, k * FREE:(k + 1) * FREE], xv[i]
                        ).then_inc(dma_sems[k], 16)
                    # Teardown, split to overlap with the output path: once
                    # PE has consumed every tile the input-DMA sems and
                    # pe_sem are quiescent, so clear them while DVE/SP run
                    # the copy+store; only out/fin teardown needs the store
                    # to have landed.
                    gp.wait_ge(pe_sem, N_TILES)
                    lo = min(s.num for s in dma_sems)
                    hi = max(s.num for s in dma_sems)
                    assert hi - lo == NBUF - 1, (lo, hi)
                    gp.sem_clear(range(lo, hi + 1))
                    gp.sem_clear(pe_sem)
                    gp.wait_ge(fin_sem, 16)
                    gp.dma_reset()
                    gp.sem_clear(out_sem)
                    gp.sem_clear(fin_sem)

                @block.tensor
                def _(te):
                    # Pack 2 row-chunks per matmul: lhsT = rhs = [A|B]
                    # ([128, 128] bf16 -> FWL kicks in), accumulating
                    # [[A'A, A'B], [B'A, B'B]] into a [128,128] PSUM tile.
                    # The two diagonal 64x64 blocks sum to the Gram
                    # contribution; off-diagonal blocks are discarded.
                    for i in range(N_TILES):
                        k = i % NBUF
                        te.wait_ge(dma_sems[k], 16 * (i // NBUF + 1))
                        for j in range(CHUNKS // 2):
                            c = xbuf[:, k * FREE + j * 2 * S:
                                     k * FREE + (j + 1) * 2 * S]
                            mm = te.matmul(
                                acc[:], c, c,
                                start=(i == 0 and j == 0),
                                stop=(i == N_TILES - 1 and j == CHUNKS // 2 - 1),
                            )
                            if j == CHUNKS // 2 - 1:
                                mm.then_inc(pe_sem, 1)

                @block.vector
                def _(v):
                    v.wait_ge(pe_sem, N_TILES)
                    v.tensor_copy(obuf[:], acc[:S, :S])
                    v.tensor_add(obuf[:], obuf[:],
                                 acc[S:, S:]).then_inc(out_sem, 1)

                @block.sync
                def _(sy):
                    sy.wait_ge(out_sem, 1)
                    sy.dma_start(g[:], obuf[:]).then_inc(fin_sem, 16)

    nc.compile()
    return nc


def get_nc():
    if "nc" not in _CACHE:
        _CACHE["nc"] = _build_nc()
    return _CACHE["nc"]


def _device_partial_grams(flat: np.ndarray, **run_kwargs) -> np.ndarray:
    """Run the SPMD bass kernel; return the 8 partial Grams [8, 64, 64]."""
    global LAST_RESULTS
    nc = get_nc()
    in_maps = [
        {"x": flat[c * N_SHARD:(c + 1) * N_SHARD]} for c in range(N_CORES)
    ]
    LAST_RESULTS = run_bass_kernel_spmd(
        nc, in_maps, core_ids=list(range(N_CORES)), **run_kwargs
    )
    return np.stack([LAST_RESULTS.results[c]["g"] for c in range(N_CORES)])


def kernel(input: np.ndarray, **run_kwargs) -> np.ndarray:
    flat = np.ascontiguousarray(
        np.asarray(input, dtype=np.float32).reshape(N_TOTAL, S)
    )
    partials = _device_partial_grams(flat, **run_kwargs)

    gram = partials.astype(np.float64).sum(axis=0)
    sq = np.diag(gram)
    dist = sq[:, None] + sq[None, :] - 2.0 * gram
    idx = np.arange(S)
    lower = idx[:, None] > idx[None, :]
    adjacent = (idx[:, None] - idx[None, :]) == 1
    per_pair = np.where(adjacent, np.maximum(0.0, MARGIN - dist), dist)
    loss = np.where(lower, per_pair, 0.0).sum() / (S * (S - 1) * 1000)
    return np.asarray(loss, dtype=np.float32)



# revision 16
# speedup vs baseline: 1.0628x; 1.0160x over previous
"""Contrastive-loss kernel for trn2 (8 NeuronCores, SPMD).

The reference loss reduces to a Gram matrix G = F.T @ F over the
flattened input F [N=524288, T=64] (128 MiB fp32), followed by a tiny
[64,64] masked margin reduction.  Each core streams a contiguous
row-shard of F (16 MiB) through SBUF as 16 tiles of 4096 rows, all
issued up-front into dedicated slots (no reuse, no flow control, so a
lagging DMA engine never throttles the healthy ones through pe_sem),
casting fp32->bf16 inline in the SWDGE DMA, and accumulates
chunk.T @ chunk matmuls (K=128, M=N=64) into one PSUM accumulator
(fp32).  The 8 partial [64,64] Grams are summed on the host, where
the masked margin reduction (negligible work) also runs.

Raw bacc (no TileContext): the kernel is a simple 3-stage pipeline
(DMA -> PE -> copy/out), and Tile's fixed preamble + end-of-kernel
drain/barrier/sem-clear machinery costs ~19us on a ~50us kernel.
Semaphore protocol:
  - dma_sem[k] (k = slot index): SWDGE incs by 16 per completed input
    DMA into slot k; PE waits 16*(round+1) before consuming.  Per-slot
    sems make the wait robust to cross-DMA completion interleaving
    (sem counts are cumulative across DMAs on one queue).
  - pe_sem: PE incs 1 on the last matmul of each tile.  With
    NBUF == N_TILES slots are never reused, so the issue loop never
    waits on it; it only gates the DVE copy and gpsimd teardown.
  - out_sem: PE-done -> DVE copies PSUM->SBUF -> incs; sync engine
    waits, stores the [64,64] result, incs fin_sem by 16.
  - gpsimd waits fin_sem, then resets DMA state and clears all sems so
    the NEFF can be re-executed (sems must be 0 at kernel entry).
"""

import numpy as np

import concourse.bacc as bacc
import concourse.mybir as mybir
from concourse.bass_utils import run_bass_kernel_spmd

MARGIN = 60000.0
S = 64                      # time steps (Gram dim)
N_TOTAL = 2 * 8 * 32 * 32 * 32   # 524288 flattened rows
N_CORES = 8
N_SHARD = N_TOTAL // N_CORES     # 65536 rows per core
P = 128                     # SBUF partitions
# Tile sizes in rows.  Uniform 4096-row tiles, except the last one is
# split in two: the PE can only start a tile after ALL its descriptors
# land, so a smaller final tile halves the serial matmul tail that runs
# after the last (often straggling) DMA packet arrives.
TILE_ROWS = [4096] * 15 + [2048, 2048]
assert sum(TILE_ROWS) == N_SHARD
TILE_FREE = [(r // P) * S for r in TILE_ROWS]      # bf16 elems/partition
TILE_OFF = [sum(TILE_FREE[:i]) for i in range(len(TILE_ROWS))]
XBUF_FREE = sum(TILE_FREE)                         # 32768 (64 KiB bf16)
N_TILES = len(TILE_ROWS)    # 17 DMA tiles, each with its own slot+sem

_CACHE = {}
LAST_RESULTS = None         # BassKernelResults of the most recent run


def _build_nc():
    nc = bacc.Bacc("TRN2", target_bir_lowering=False, debug=False,
                   num_devices=N_CORES)
    x = nc.dram_tensor("x", [N_SHARD, S], mybir.dt.float32,
                       kind="ExternalInput")
    g = nc.dram_tensor("g", [S, S], mybir.dt.float32, kind="ExternalOutput")
    n_big = sum(1 for r in TILE_ROWS if r == 4096)
    xv_big = x[:n_big * 4096].rearrange("(n p r) c -> n p (r c)", p=P, r=32)
    xv_small = x[n_big * 4096:].rearrange("(n p r) c -> n p (r c)", p=P, r=16)

    def tile_src(i):
        return xv_big[i] if i < n_big else xv_small[i - n_big]

    with (
        nc.sbuf_tensor("xbuf", [P, XBUF_FREE], mybir.dt.bfloat16) as xbuf,
        nc.psum_tensor("acc", [2 * S, 2 * S], mybir.dt.float32) as acc,
        nc.sbuf_tensor("obuf", [S, S], mybir.dt.float32) as obuf,
        nc.semaphore("pe_sem") as pe_sem,
        nc.semaphore("out_sem") as out_sem,
        nc.semaphore("fin_sem") as fin_sem,
    ):
        dma_sems = []
        import contextlib
        with contextlib.ExitStack() as stack:
            for k in range(N_TILES):
                dma_sems.append(stack.enter_context(
                    nc.semaphore(f"dma_sem{k}")))
            all_sems = [pe_sem, out_sem, fin_sem] + dma_sems

            with nc.Block() as block:

                @block.gpsimd
                def _(gp):
                    for i in range(N_TILES):
                        gp.dma_start(
                            xbuf[:, TILE_OFF[i]:TILE_OFF[i] + TILE_FREE[i]],
                            tile_src(i),
                        ).then_inc(dma_sems[i], 16)
                    # Teardown, split to overlap with the output path: once
                    # PE has consumed every tile the input-DMA sems and
                    # pe_sem are quiescent, so clear them while DVE/SP run
                    # the copy+store; only out/fin teardown needs the store
                    # to have landed.
                    gp.wait_ge(pe_sem, N_TILES)
                    lo = min(s.num for s in dma_sems)
                    hi = max(s.num for s in dma_sems)
                    assert hi - lo == N_TILES - 1, (lo, hi)
                    gp.sem_clear(range(lo, hi + 1))
                    gp.sem_clear(pe_sem)
                    gp.wait_ge(fin_sem, 16)
                    gp.dma_reset()
                    gp.sem_clear(out_sem)
                    gp.sem_clear(fin_sem)

                @block.tensor
                def _(te):
                    # Pack 2 row-chunks per matmul: lhsT = rhs = [A|B]
                    # ([128, 128] bf16 -> FWL kicks in), accumulating
                    # [[A'A, A'B], [B'A, B'B]] into a [128,128] PSUM tile.
                    # The two diagonal 64x64 blocks sum to the Gram
                    # contribution; off-diagonal blocks are discarded.
                    for i in range(N_TILES):
                        te.wait_ge(dma_sems[i], 16)
                        pairs = TILE_FREE[i] // (2 * S)
                        for j in range(pairs):
                            c = xbuf[:, TILE_OFF[i] + j * 2 * S:
                                     TILE_OFF[i] + (j + 1) * 2 * S]
                            mm = te.matmul(
                                acc[:], c, c,
                                start=(i == 0 and j == 0),
                                stop=(i == N_TILES - 1 and j == pairs - 1),
                            )
                            if j == pairs - 1:
                                mm.then_inc(pe_sem, 1)

                @block.vector
                def _(v):
                    v.wait_ge(pe_sem, N_TILES)
                    v.tensor_copy(obuf[:], acc[:S, :S])
                    v.tensor_add(obuf[:], obuf[:],
                                 acc[S:, S:]).then_inc(out_sem, 1)

                @block.sync
                def _(sy):
                    sy.wait_ge(out_sem, 1)
                    sy.dma_start(g[:], obuf[:]).then_inc(fin_sem, 16)

    nc.compile()
    return nc


def get_nc():
    if "nc" not in _CACHE:
        _CACHE["nc"] = _build_nc()
    return _CACHE["nc"]


def _device_partial_grams(flat: np.ndarray, **run_kwargs) -> np.ndarray:
    """Run the SPMD bass kernel; return the 8 partial Grams [8, 64, 64]."""
    global LAST_RESULTS
    nc = get_nc()
    in_maps = [
        {"x": flat[c * N_SHARD:(c + 1) * N_SHARD]} for c in range(N_CORES)
    ]
    LAST_RESULTS = run_bass_kernel_spmd(
        nc, in_maps, core_ids=list(range(N_CORES)), **run_kwargs
    )
    return np.stack([LAST_RESULTS.results[c]["g"] for c in range(N_CORES)])


def kernel(input: np.ndarray, **run_kwargs) -> np.ndarray:
    flat = np.ascontiguousarray(
        np.asarray(input, dtype=np.float32).reshape(N_TOTAL, S)
    )
    partials = _device_partial_grams(flat, **run_kwargs)

    gram = partials.astype(np.float64).sum(axis=0)
    sq = np.diag(gram)
    dist = sq[:, None] + sq[None, :] - 2.0 * gram
    idx = np.arange(S)
    lower = idx[:, None] > idx[None, :]
    adjacent = (idx[:, None] - idx[None, :]) == 1
    per_pair = np.where(adjacent, np.maximum(0.0, MARGIN - dist), dist)
    loss = np.where(lower, per_pair, 0.0).sum() / (S * (S - 1) * 1000)
    return np.asarray(loss, dtype=np.float32)

